# revision 1
# baseline (speedup 1.0000x reference)
"""CPI_DGLLife kernel for 8 Trainium2 NeuronCores (SPMD).

GCN over a 65536-node graph + protein conv1d branch + CPI head.
Sharding: data-parallel over the 512-graph batch (64 graphs / core).
Each core: full h0 table build (replicated), dma_gather edge aggregation
for its dst nodes, fp32r conv stack for its 64 proteins.
"""
import sys
sys.path.insert(0, "/opt/trn_rl_repo")
import contextlib
import numpy as np

import concourse.bass as bass
import concourse.bacc as bacc
import concourse.tile as tile
from concourse import mybir
from concourse.bass_utils import run_bass_kernel_spmd
from concourse.masks import make_identity

dt = mybir.dt
AF = mybir.ActivationFunctionType
ALU = mybir.AluOpType
AX = mybir.AxisListType

P = 128
N, E, B, L = 65536, 262144, 512, 1000
IN_DIM, HID, VOCAB = 74, 128, 25
CHANNELS = [HID, 96, 128, IN_DIM, HID]
NCORES = 8
GPC = B // NCORES              # graphs per core = 64
PPC = GPC                      # proteins per core = 64
# h0 tables: 512-aligned split, local idx = node - base + 1, row 0 = zeros
TBL_BASES = [0, 32256, 64512]
TBL_NNODES = [32256, 32256, 1024]
TBL_ROWS = [n + 1 for n in TBL_NNODES]
TOK_BUDGET = 4096              # max tokens per dma_gather instruction
LCONV = 1002                   # 1000 + 2 guard cols


# ------------------------------------------------------------------ host prep
def _host_prep(inputs):
    graph_ids = np.asarray(inputs["graph_ids"])
    src = np.concatenate([np.asarray(inputs["edge_src"]).astype(np.int64),
                          np.arange(N, dtype=np.int64)])
    dst = np.concatenate([np.asarray(inputs["edge_dst"]).astype(np.int64),
                          np.arange(N, dtype=np.int64)])
    deg_out = np.bincount(src, minlength=N).astype(np.float32)
    deg_in = np.bincount(dst, minlength=N).astype(np.float32)

    core_node_lo = np.searchsorted(graph_ids, np.arange(0, B + 1, GPC))
    ncore_nodes = core_node_lo[1:] - core_node_lo[:-1]
    NT = int(np.ceil(ncore_nodes.max() / P))  # tiles per core (uniform)
    NPAD = NT * P

    # per-core degree-sorted node permutation (padded with -1)
    perm = np.full((NCORES, NPAD), -1, np.int64)
    for c in range(NCORES):
        lo, hi = int(core_node_lo[c]), int(core_node_lo[c + 1])
        order = np.argsort(-deg_in[lo:hi], kind="stable") + lo
        perm[c, :hi - lo] = order

    # deg_in per perm position (pad 1.0), laid out [P, NT] (p, t)
    deg_in_perm = np.ones((NCORES, NPAD), np.float32)
    m = perm >= 0
    deg_in_perm[m] = deg_in[perm[m]]
    deg_in_perm = deg_in_perm.reshape(NCORES, NT, P).transpose(0, 2, 1).copy()

    # S tiles: [NT, P, GPC] graph membership of permuted nodes
    S = np.zeros((NCORES, NT, P, GPC), np.float32)
    for c in range(NCORES):
        pm = perm[c]
        valid = pm >= 0
        g = graph_ids[pm[valid]] - c * GPC
        tt = np.arange(NPAD)[valid] // P
        pp = np.arange(NPAD)[valid] % P
        S[c, tt, pp, g] = 1.0

    # node -> (core, tile-position) in permuted order
    pos_of = np.full(N, -1, np.int64)
    core_of = np.full(N, -1, np.int64)
    for c in range(NCORES):
        pm = perm[c]
        v = pm >= 0
        pos_of[pm[v]] = np.arange(NPAD)[v]
        core_of[pm[v]] = c

    # table id + local row of each node (as gather source)
    tbl_of = np.digitize(np.arange(N), TBL_BASES[1:])
    loc_of = (np.arange(N) - np.asarray(TBL_BASES)[tbl_of] + 1).astype(np.int64)

    # edge placement: core/tile/lane from dst, table/local from src
    ec = core_of[dst]
    et = pos_of[dst] // P
    ep = pos_of[dst] % P
    etbl = tbl_of[src]
    eloc = loc_of[src]

    # slot index within (core, tile, lane, table) group
    key = (((ec * NT + et) * P + ep) * 3 + etbl)
    order = np.argsort(key, kind="stable")
    ks = key[order]
    starts = np.r_[0, np.flatnonzero(np.diff(ks)) + 1]
    grp_len = np.diff(np.r_[starts, E + N])
    slot_sorted = np.arange(E + N) - np.repeat(starts, grp_len)
    slot = np.empty(E + N, np.int64)
    slot[order] = slot_sorted
    # counts per (c, t, p, T) -> kmax per (t, T) across cores/lanes
    cnt = np.zeros(NCORES * NT * P * 3, np.int64)
    uk, uc = np.unique(ks, return_counts=True)
    cnt[uk] = uc
    cnt = cnt.reshape(NCORES, NT, P, 3)
    kmax = cnt.max(axis=2).max(axis=0)  # [NT, 3]

    # gather token schedule per table: tiles packed into instructions
    sched = []  # per table: list of instruction = list of (tile, k)
    for T in range(3):
        instrs, cur, tok = [], [], 0
        for t in range(NT):
            k = int(kmax[t, T])
            if k == 0:
                continue
            if tok + k * P > TOK_BUDGET and cur:
                instrs.append(cur)
                cur, tok = [], 0
            cur.append((t, k))
            tok += k * P
        if cur:
            instrs.append(cur)
        sched.append(instrs)

    # token offset of each tile inside its table stream
    tile_off = np.full((3, NT), 0, np.int64)
    tok_total = [0, 0, 0]
    for T in range(3):
        off = 0
        for ins in sched[T]:
            for (t, k) in ins:
                tile_off[T, t] = off
                off += k * P
        tok_total[T] = max(off, 128)

    idx_flat = [np.zeros((NCORES, tok_total[T]), np.int16) for T in range(3)]
    tok_pos = tile_off[etbl, et] + slot * P + ep
    for T in range(3):
        mT = etbl == T
        idx_flat[T][ec[mT], tok_pos[mT]] = eloc[mT].astype(np.int16)

    def wrap(a):  # token-major -> wrapped [128, tokens//16]
        ncol = a.shape[1] // 16
        w = a.reshape(a.shape[0], ncol, 16).transpose(0, 2, 1)
        return np.ascontiguousarray(np.tile(w, (1, 8, 1)))

    idx_wrapped = [wrap(ix) for ix in idx_flat]

    # per-token deg_out in gather-output layout [128, tokens//128]
    nf = np.asarray(inputs["node_feats"], np.float32)
    tabs = []
    dtok = []
    for T in range(3):
        tb = np.zeros((TBL_ROWS[T], P), np.float32)
        nn = TBL_NNODES[T]
        tb[1:1 + nn, :IN_DIM] = nf[TBL_BASES[T]:TBL_BASES[T] + nn]
        tabs.append(tb)
        d = np.ones((NCORES, tok_total[T]), np.float32)
        mT = etbl == T
        d[ec[mT], tok_pos[mT]] = deg_out[src[mT]]
        dtok.append(np.ascontiguousarray(
            d.reshape(NCORES, tok_total[T] // P, P).transpose(0, 2, 1)))

    # one-hot proteins grouped 4/DMA: [PPC//4, 128, LCONV], p = g*4+s
    seq = np.asarray(inputs["protein_seq"]).reshape(NCORES, PPC, L)
    oh = np.zeros((NCORES, PPC, 32, LCONV), np.float32)
    iot = np.arange(VOCAB)[None, None, :, None]
    oh[:, :, :VOCAB, 1:1 + L] = (seq[:, :, None, :] == iot)
    oh = np.ascontiguousarray(
        oh.reshape(NCORES, PPC // 4, 4 * 32, LCONV))

    shared = {
        "tab0": tabs[0], "tab1": tabs[1], "tab2": tabs[2],
        "W_gc": np.asarray(inputs["W_gc"], np.float32),
        "b_gc": np.asarray(inputs["b_gc"], np.float32).reshape(HID, 1),
        "W_ro_in": np.asarray(inputs["W_ro_in"], np.float32),
        "b_ro_in": np.asarray(inputs["b_ro_in"], np.float32).reshape(HID, 1),
        "W_ro_out": np.asarray(inputs["W_ro_out"], np.float32),
        "b_ro_out": np.asarray(inputs["b_ro_out"], np.float32).reshape(HID, 1),
        "Wc1": np.asarray(inputs["Wc1"], np.float32),
        "bc1": np.asarray(inputs["bc1"], np.float32).reshape(HID, 1),
        "Wc2": np.asarray(inputs["Wc2"], np.float32),
        "bc2": np.asarray(inputs["bc2"], np.float32).reshape(HID, 1),
        "embedT": np.ascontiguousarray(
            np.asarray(inputs["embed"], np.float32).T),       # [HID, 25]
        "Wf1_r": np.ascontiguousarray(
            np.asarray(inputs["Wf1"], np.float32).reshape(2, HID, 2 * HID)),
        "bf1_r": np.ascontiguousarray(
            np.asarray(inputs["bf1"], np.float32).reshape(2, HID, 1)),
        "Wf2_r": np.ascontiguousarray(
            np.asarray(inputs["Wf2"], np.float32).reshape(2, HID, 1)),
        "bf2": np.asarray(inputs["bf2"], np.float32).reshape(1, 1),
    }
    for l in range(4):
        K = np.asarray(inputs["K%d" % (l + 1)], np.float32)  # [o, i, 3]
        shared["K%dT" % (l + 1)] = np.ascontiguousarray(
            K.transpose(1, 2, 0))                            # [i, 3, o]
        shared["cb%d" % (l + 1)] = np.asarray(
            inputs["cb%d" % (l + 1)], np.float32).reshape(-1, 1)

    percore = []
    for c in range(NCORES):
        percore.append({
            "deg_in_perm": np.ascontiguousarray(deg_in_perm[c]),
            "S": np.ascontiguousarray(S[c]),
            "onehot": np.ascontiguousarray(oh[c]),
            "ix0": idx_wrapped[0][c],
            "ix1": idx_wrapped[1][c],
            "ix2": idx_wrapped[2][c],
            "dtok0": dtok[0][c], "dtok1": dtok[1][c], "dtok2": dtok[2][c],
        })
    meta = dict(NT=NT, sched=sched, tok_total=tok_total)
    return shared, percore, meta


# --------------------------------------------------------------- device build
def _build(shared, meta):
    NT = meta["NT"]
    sched = meta["sched"]
    tok_total = meta["tok_total"]

    nc = bacc.Bacc("TRN2", target_bir_lowering=False, debug=False,
                   num_devices=NCORES, num_swdge_queues=4)
    f32, f32r, i16 = dt.float32, dt.float32r, dt.int16

    D = {k: nc.dram_tensor(k, list(v.shape), dt.from_np(v.dtype),
                           kind="ExternalInput")
         for k, v in shared.items()}
    D["deg_in_perm"] = nc.dram_tensor("deg_in_perm", [P, NT], f32,
                                      kind="ExternalInput")
    D["S"] = nc.dram_tensor("S", [NT, P, GPC], f32, kind="ExternalInput")
    D["onehot"] = nc.dram_tensor("onehot", [PPC // 4, P, LCONV], f32,
                                 kind="ExternalInput")
    for T in range(3):
        D["ix%d" % T] = nc.dram_tensor("ix%d" % T, [P, tok_total[T] // 16],
                                       i16, kind="ExternalInput")
    tabs = [D["tab%d" % T] for T in range(3)]
    for T in range(3):
        D["dtok%d" % T] = nc.dram_tensor("dtok%d" % T, [P, tok_total[T] // P],
                                         f32, kind="ExternalInput")
    out_d = nc.dram_tensor("out", [1, GPC], f32, kind="ExternalOutput")

    with tile.TileContext(nc) as tc, contextlib.ExitStack() as ctx:
        wp = ctx.enter_context(tc.tile_pool(name="wp", bufs=1))
        h0p = ctx.enter_context(tc.tile_pool(name="h0p", bufs=3))
        gp = ctx.enter_context(tc.tile_pool(name="gp", bufs=1))
        accp = ctx.enter_context(tc.tile_pool(name="accp", bufs=1))
        cvp = ctx.enter_context(tc.tile_pool(name="cvp", bufs=2))
        gnp = ctx.enter_context(tc.tile_pool(name="gnp", bufs=3))
        pcv = ctx.enter_context(tc.tile_pool(name="pcv", bufs=4, space="PSUM"))
        pgn = ctx.enter_context(tc.tile_pool(name="pgn", bufs=2, space="PSUM"))
        ps1 = ctx.enter_context(tc.tile_pool(name="ps1", bufs=1, space="PSUM"))

        # ---------------- setup: weights to SBUF
        def ld(name, shape, dtype=f32, src=None, tag=None):
            t = wp.tile(shape, dtype, tag=tag or name)
            ap = D[name][:] if src is None else src
            if dtype == f32r:
                ap = ap.bitcast(f32r)
            nc.sync.dma_start(out=t[:], in_=ap)
            return t

        W_gc = ld("W_gc", [IN_DIM, HID], f32r)
        b_gc = ld("b_gc", [HID, 1])
        W_ri = ld("W_ro_in", [HID, HID], f32r); b_ri = ld("b_ro_in", [HID, 1])
        W_ro = ld("W_ro_out", [HID, HID], f32r); b_ro = ld("b_ro_out", [HID, 1])
        Wc1 = ld("Wc1", [HID, HID], f32r); bc1 = ld("bc1", [HID, 1])
        Wc2 = ld("Wc2", [HID, HID], f32r); bc2 = ld("bc2", [HID, 1])
        Wf1 = ld("Wf1_r", [HID, 2, 2 * HID],
                 src=D["Wf1_r"][:].rearrange("k h m -> h k m"))
        bf1 = ld("bf1_r", [HID, 2, 1],
                 src=D["bf1_r"][:].rearrange("k h o -> h k o"))
        Wf2 = ld("Wf2_r", [HID, 2, 1],
                 src=D["Wf2_r"][:].rearrange("k h o -> h k o"))
        bf2 = ld("bf2", [1, 1])
        embT = ld("embedT", [HID, VOCAB], f32r)
        KT = [ld("K%dT" % (l + 1), [CHANNELS[l], 3, CHANNELS[l + 1]], f32r)
              for l in range(4)]
        cb = [ld("cb%d" % (l + 1), [CHANNELS[l + 1], 1]) for l in range(4)]
        Sg = ld("S", [P, NT, GPC], f32r,
                src=D["S"][:].rearrange("t p g -> p t g"))
        ixs = [ld("ix%d" % T, [P, tok_total[T] // 16], i16) for T in range(3)]
        dginp = ld("deg_in_perm", [P, NT])
        dts = [ld("dtok%d" % T, [P, tok_total[T] // P]) for T in range(3)]

        xb = []
        for l in range(3):
            pair = []
            for j in range(2):
                t = wp.tile([CHANNELS[l + 1], LCONV], f32r,
                            tag="xb%d_%d" % (l, j))
                nc.vector.memset(t[:, 0:1].bitcast(dt.float32), 0.0)
                nc.vector.memset(t[:, LCONV - 1:LCONV].bitcast(dt.float32),
                                 0.0)
                pair.append(t)
            xb.append(pair)

        ident = wp.tile([P, P], f32, tag="ident")
        make_identity(nc, ident[:])
        identr = wp.tile([P, P], f32r, tag="identr")
        nc.vector.tensor_copy(identr[:], ident[:])

        # rsqrt factors: w = sqrt(1/deg) per gather token / per dst lane
        for T in range(3):
            nc.vector.reciprocal(dts[T][:], dts[T][:])
            nc.scalar.sqrt(dts[T][:], dts[T][:])
        rdgi = wp.tile([P, NT], f32, tag="rdgi")
        nc.vector.reciprocal(rdgi[:], dginp[:])
        nc.scalar.sqrt(rdgi[:], rdgi[:])

        # M1rep[32s:32s+25, t, :] = embed @ K1_t^T replicated at 4 offsets
        M1rep = wp.tile([P, 3, CHANNELS[1]], f32r, tag="m1rep")
        for t in range(3):
            pm = ps1.tile([VOCAB, CHANNELS[1]], f32, space="PSUM", tag="ps1a")
            nc.tensor.matmul(pm[:], embT[:], KT[0][:, t, :], start=True,
                             stop=True)
            nc.scalar.copy(M1rep[:VOCAB, t, :], pm[:])
        for srow in range(1, 4):
            nc.sync.dma_start(out=M1rep[32 * srow:32 * srow + VOCAB, :, :],
                              in_=M1rep[:VOCAB, :, :])

        # ---------------- interleaved: conv proteins + gather groups
        acc = {}

        def emit_group(grp, after_protein=None):
            ohg = cvp.tile([P, LCONV], f32r, tag="ohg")
            nc.sync.dma_start(out=ohg[:], in_=D["onehot"][grp].bitcast(f32r))
            for srow in range(4):
                p = grp * 4 + srow
                b0 = 32 * srow
                xs = None
                for l in range(4):
                    cin, cout = CHANNELS[l], CHANNELS[l + 1]
                    for cchunk in range(2):
                        c0 = cchunk * 500
                        pps = pcv.tile([cout, 500], f32, space="PSUM",
                                       tag="cps")
                        for tap in range(3):
                            if l == 0:
                                lhsT = M1rep[b0:b0 + VOCAB, tap, :]
                                rhs = ohg[b0:b0 + VOCAB,
                                          c0 + tap:c0 + tap + 500]
                                tpos = (96, 0) if srow == 3 else None
                            else:
                                lhsT = KT[l][:, tap, :]
                                rhs = xs[:cin, c0 + tap:c0 + tap + 500]
                                tpos = None
                            nc.tensor.matmul(pps[:], lhsT, rhs,
                                             start=(tap == 0), stop=(tap == 2),
                                             tile_position=tpos)
                        if l < 3:
                            nc.scalar.activation(
                                xb[l][p % 2][:, 1 + c0:1 + c0 + 500],
                                pps[:], AF.Relu, bias=cb[l][:])
                        else:
                            nc.vector.reduce_max(
                                out=chunkmax[:, cchunk, p:p + 1],
                                in_=pps[:, :500], axis=AX.X)
                    if l < 3:
                        xs = xb[l][p % 2]
                if after_protein is not None:
                    after_protein(p)

        gjobs = []
        for T in range(3):
            off = 0
            for ins in sched[T]:
                gjobs.append((T, off, ins))
                off += sum(k * P for (_, k) in ins)

        def emit_gather(job, qn):
            T, off, ins = job
            ntok = sum(k * P for (_, k) in ins)
            g = gp.tile([P, ntok // P, P], f32, tag="g%d" % (qn % 6))
            nc.gpsimd.dma_gather(
                out_ap=g[:], in_ap=tabs[T][:],
                idxs_ap=ixs[T][:, off // 16:(off + ntok) // 16],
                num_idxs=ntok, num_idxs_reg=ntok, elem_size=P,
                single_packet=False, queue_num=qn % 4)
            blk0 = off // P
            nc.vector.tensor_tensor(
                out=g[:, :, :IN_DIM],
                in0=g[:, :, :IN_DIM],
                in1=dts[T][:, blk0:blk0 + ntok // P, None]
                    .to_broadcast([P, ntok // P, IN_DIM]),
                op=ALU.mult)
            boff = 0
            for (t, k) in ins:
                view = g[:, boff:boff + k, :IN_DIM].rearrange("p k d -> p d k")
                if t not in acc:
                    a = accp.tile([P, IN_DIM], f32, tag="acc%d" % t)
                    acc[t] = a
                    nc.vector.tensor_reduce(out=a[:], in_=view, axis=AX.X,
                                            op=ALU.add)
                else:
                    tmp = gp.tile([P, IN_DIM], f32, tag="rtmp")
                    nc.vector.tensor_reduce(out=tmp[:], in_=view, axis=AX.X,
                                            op=ALU.add)
                    nc.vector.tensor_add(out=acc[t][:], in0=acc[t][:],
                                         in1=tmp[:])
                boff += k

        pmax = wp.tile([P, PPC], f32, tag="pmax")
        chunkmax = wp.tile([P, 2, PPC], f32, tag="chunkmax")
        gq = list(gjobs)
        qst = [0]

        def drain(p):
            while gq and len(gq) > (PPC - 1 - p) * len(gjobs) // PPC:
                emit_gather(gq.pop(0), qst[0])
                qst[0] += 1

        for grp in range(PPC // 4):
            emit_group(grp, after_protein=drain)
        qn = qst[0]
        while gq:
            emit_gather(gq.pop(0), qn)
            qn += 1
        # pmax = relu(max(chunk maxes) + cb4)
        mxt = wp.tile([P, PPC], f32, tag="mxt")
        nc.vector.tensor_reduce(out=mxt[:],
                                in_=chunkmax[:].rearrange("p c q -> p q c"),
                                axis=AX.X, op=ALU.max)
        nc.scalar.activation(pmax[:], mxt[:], AF.Relu, bias=cb[3][:])
        # scale by rsqrt(deg_in)
        for t in range(NT):
            nc.vector.tensor_scalar_mul(acc[t][:], acc[t][:],
                                        rdgi[:, t:t + 1])

        # ---------------- GNN matmul chain (fp32)
        hg_ps = ps1.tile([GPC, HID], f32, space="PSUM", tag="hgps")
        for t in range(NT):
            tp = pgn.tile([IN_DIM, P], f32, space="PSUM", tag="gps")
            nc.tensor.transpose(tp[:], acc[t][:], ident[:])
            aggT = gnp.tile([IN_DIM, P], f32r, tag="aggT")
            nc.scalar.copy(aggT[:], tp[:])
            hps = pgn.tile([HID, P], f32, space="PSUM", tag="gps")
            nc.tensor.matmul(hps[:], W_gc[:], aggT[:], start=True, stop=True)
            h = gnp.tile([HID, P], f32r, tag="h")
            nc.scalar.activation(h[:], hps[:], AF.Relu, bias=b_gc[:])
            x1ps = pgn.tile([HID, P], f32, space="PSUM", tag="gps")
            nc.tensor.matmul(x1ps[:], W_ri[:], h[:], start=True, stop=True)
            x1 = gnp.tile([HID, P], f32r, tag="x1")
            nc.scalar.activation(x1[:], x1ps[:], AF.Identity, bias=b_ri[:])
            x2ps = pgn.tile([HID, P], f32, space="PSUM", tag="gps")
            nc.tensor.matmul(x2ps[:], W_ro[:], x1[:], start=True, stop=True)
            x2 = gnp.tile([HID, P], f32r, tag="x2")
            nc.scalar.activation(x2[:], x2ps[:], AF.Identity, bias=b_ro[:])
            x2t = pgn.tile([P, HID], f32r, space="PSUM", tag="gps")
            nc.tensor.transpose(x2t[:], x2[:], identr[:])
            x2n = gnp.tile([P, HID], f32r, tag="x2n")
            nc.scalar.copy(x2n[:], x2t[:])
            nc.tensor.matmul(hg_ps[:], Sg[:, t, :], x2n[:],
                             start=(t == 0), stop=(t == NT - 1),
                             skip_group_check=True)
        hgT = wp.tile([GPC, HID], f32, tag="hgT")
        nc.scalar.activation(hgT[:], hg_ps[:], AF.Relu)
        hgt_ps = pgn.tile([HID, GPC], f32, space="PSUM", tag="gps")
        nc.tensor.transpose(hgt_ps[:], hgT[:], ident[:GPC, :GPC])
        hg = wp.tile([HID, GPC], f32r, tag="hg")
        nc.scalar.copy(hg[:], hgt_ps[:])
        # compound FC
        c1ps = pgn.tile([HID, GPC], f32, space="PSUM", tag="gps")
        nc.tensor.matmul(c1ps[:], Wc1[:], hg[:], start=True, stop=True)
        cv1 = wp.tile([HID, GPC], f32r, tag="cv1")
        nc.scalar.activation(cv1[:], c1ps[:], AF.Relu, bias=bc1[:])
        c2ps = pgn.tile([HID, GPC], f32, space="PSUM", tag="gps")
        nc.tensor.matmul(c2ps[:], Wc2[:], cv1[:], start=True, stop=True)
        cv2 = wp.tile([HID, GPC], f32, tag="cv2")
        nc.scalar.activation(cv2[:], c2ps[:], AF.Relu, bias=bc2[:])
        # head: z = [cv2; pmax]
        zin = [cv2, pmax]
        z2 = []
        for mc in range(2):
            zps = pgn.tile([HID, GPC], f32, space="PSUM", tag="gps")
            for kc in range(2):
                nc.tensor.matmul(zps[:], Wf1[:, kc, mc * HID:(mc + 1) * HID],
                                 zin[kc][:, :GPC], start=(kc == 0),
                                 stop=(kc == 1))
            zt = wp.tile([HID, GPC], f32, tag="z2_%d" % mc)
            nc.scalar.activation(zt[:], zps[:], AF.Relu, bias=bf1[:, mc, :])
            z2.append(zt)
        ops = ps1.tile([1, GPC], f32, space="PSUM", tag="ps1a")
        for kc in range(2):
            nc.tensor.matmul(ops[:], Wf2[:, kc, :], z2[kc][:],
                             start=(kc == 0), stop=(kc == 1))
        ot = wp.tile([1, GPC], f32, tag="ot")
        nc.scalar.activation(ot[:], ops[:], AF.Sigmoid, bias=bf2[:1, :])
        nc.sync.dma_start(out=out_d[:], in_=ot[:])

    nc.compile()
    return nc


def kernel(**inputs):
    shared, percore, meta = _host_prep(inputs)
    nc = _build(shared, meta)
    in_maps = []
    for c in range(NCORES):
        m = dict(shared)
        m.update(percore[c])
        in_maps.append(m)
    res = run_bass_kernel_spmd(nc, in_maps, list(range(NCORES)))
    out = np.concatenate([res.results[c]["out"].reshape(GPC)
                          for c in range(NCORES)])
    return out.reshape(B, 1).astype(np.float32)


if __name__ == "__main__":
    sys.path.insert(0, "/root/problem")
    import jax
    import reference
    with jax.default_device(jax.devices("cpu")[0]):
        inputs = {k: np.asarray(v) for k, v in reference.setup_inputs().items()}
        exp = np.asarray(reference.reference(**inputs))
    got = kernel(**inputs)
    err = np.abs(got - exp).max()
    rel = err / max(np.abs(exp).max(), 1e-9)
    print("max abs err:", err, " rel:", rel)



# revision 20
# speedup vs baseline: 3.6700x; 3.6700x over previous
"""CPI_DGLLife kernel for 8 Trainium2 NeuronCores (SPMD), v2.

GCN over a 65536-node graph + protein conv1d branch + CPI head.
Sharding: data-parallel over the 512-graph batch (64 graphs / core).

v2 design (all matmuls bf16):
- conv layer 1 via stacked-tap one-hot (75-row rhs, 1 matmul per chunk);
  biases folded into matmuls via ones-rows (except layer 3, fused on DVE).
- GCN aggregation: dense dma_gather from a per-core compacted src table
  (rsqrt(deg_out) prescaled), segment-sum via one-hot M-matrix matmuls,
  self-loops added from a contiguous slab via DVE.
- node readout (two affine layers, no relu) folded into one matrix on
  host and applied per-graph after the S-matmul sum.
"""
import sys
sys.path.insert(0, "/opt/trn_rl_repo")
import contextlib
import numpy as np
import ml_dtypes

import concourse.bass as bass
import concourse.bacc as bacc
import concourse.tile as tile
from concourse import mybir
from concourse.bass_utils import run_bass_kernel_spmd
from concourse.masks import make_identity

dt = mybir.dt
AF = mybir.ActivationFunctionType
ALU = mybir.AluOpType
AX = mybir.AxisListType

P = 128
N, E, B, L = 65536, 262144, 512, 1000
IN_DIM, HID, VOCAB = 74, 128, 25
CHANNELS = [HID, 96, 128, IN_DIM, HID]
NCORES = 8
GPC = B // NCORES              # graphs per core = 64
PPC = GPC                      # proteins per core = 64
TOK_PER_G = 4096               # tokens per dma_gather instruction
BF16 = ml_dtypes.bfloat16


def _bf(a):
    return np.ascontiguousarray(np.asarray(a, np.float32).astype(BF16))


# ------------------------------------------------------------------ host prep
def _host_prep(inputs):
    graph_ids = np.asarray(inputs["graph_ids"])
    esrc = np.asarray(inputs["edge_src"]).astype(np.int64)
    edst = np.asarray(inputs["edge_dst"]).astype(np.int64)
    src_all = np.concatenate([esrc, np.arange(N, dtype=np.int64)])
    dst_all = np.concatenate([edst, np.arange(N, dtype=np.int64)])
    deg_out = np.bincount(src_all, minlength=N).astype(np.float32)
    deg_in = np.bincount(dst_all, minlength=N).astype(np.float32)
    rs_out = 1.0 / np.sqrt(deg_out)
    rs_in = 1.0 / np.sqrt(deg_in)

    nf = np.asarray(inputs["node_feats"], np.float32)
    nf_scaled = nf * rs_out[:, None]           # rows for gather table

    core_node_lo = np.searchsorted(graph_ids, np.arange(0, B + 1, GPC))
    ncore_nodes = core_node_lo[1:] - core_node_lo[:-1]
    NT = int(np.ceil(ncore_nodes.max() / P))
    NPAD = NT * P

    # ---- per-core edge streams (natural dst order), compacted src tables
    ecore = np.searchsorted(core_node_lo[1:], edst, side="right")
    order = np.argsort(ecore * np.int64(N) + edst, kind="stable")
    es, ed, ec = esrc[order], edst[order], ecore[order]
    core_e_lo = np.searchsorted(ec, np.arange(NCORES + 1))

    # per (core, tile) counts -> uniform nblk per tile (max over cores)
    cnt = np.zeros((NCORES, NT), np.int64)
    for c in range(NCORES):
        lo, hi = core_e_lo[c], core_e_lo[c + 1]
        pos = ed[lo:hi] - core_node_lo[c]
        np.add.at(cnt[c], pos // P, 1)
    nblk_t = np.maximum(1, np.ceil(cnt.max(axis=0) / P).astype(np.int64))
    NBLK = int(nblk_t.sum())
    TOKS = int(np.ceil(NBLK * P / TOK_PER_G) * TOK_PER_G)
    NG = TOKS // TOK_PER_G
    NBLK_PAD = TOKS // P
    blk0_t = np.concatenate([[0], np.cumsum(nblk_t)])  # block offset per tile

    tabs, idxs, Ms, slabs, rdgis, sgts, ngs = [], [], [], [], [], [], []
    TABROWS = 32768
    for c in range(NCORES):
        lo, hi = core_e_lo[c], core_e_lo[c + 1]
        s, d = es[lo:hi], ed[lo:hi] - core_node_lo[c]
        uniq, inv = np.unique(s, return_inverse=True)
        assert len(uniq) + 1 <= TABROWS, len(uniq)
        tab = np.zeros((TABROWS, P), np.float32)
        tab[1:1 + len(uniq), :IN_DIM] = nf_scaled[uniq]
        tabs.append(_bf(tab))

        # token stream: per tile, its edges at block offset blk0_t[t]
        tok_idx = np.zeros(TOKS, np.int16)
        Mm = np.zeros((NBLK * P,), np.int64)  # dst lane per token (-1 = pad)
        Mm[:] = -1
        t_of = d // P
        lane = d % P
        # position within tile = running index among that tile's edges
        o2 = np.argsort(t_of, kind="stable")
        t_sorted = t_of[o2]
        starts = np.searchsorted(t_sorted, np.arange(NT))
        for t in range(NT):
            a, b = starts[t], starts[t] + cnt[c][t]
            sel = o2[a:b]
            base = blk0_t[t] * P
            tok_idx[base:base + len(sel)] = (inv[sel] + 1).astype(np.int16)
            Mm[base:base + len(sel)] = lane[sel]
        idxs.append(tok_idx)

        # M matrices [128 tok, NBLK_PAD, 128 dst] bf16 (token-part. major)
        M = np.zeros((P, NBLK_PAD, P), np.float32)
        tok = np.arange(NBLK * P)
        valid = Mm >= 0
        M[tok[valid] % P, tok[valid] // P, Mm[valid]] = 1.0
        Ms.append(_bf(M))

        # self slab [128, NT, 128]: X[n]*rs_out[n]*rs_in[n] in natural order
        slab = np.zeros((NPAD, P), np.float32)
        nn = int(ncore_nodes[c])
        nodes = np.arange(core_node_lo[c], core_node_lo[c] + nn)
        slab[:nn, :IN_DIM] = nf[nodes] * (rs_out[nodes] * rs_in[nodes])[:, None]
        slabs.append(_bf(slab.reshape(NT, P, P).transpose(1, 0, 2)))

        rdgi = np.ones((NPAD,), np.float32)
        rdgi[:nn] = rs_in[nodes]
        rdgis.append(np.ascontiguousarray(
            rdgi.reshape(NT, P).T.astype(np.float32)))

        # S^T tiles [128 node-lane, NT, 64 graphs]
        S = np.zeros((NPAD, GPC), np.float32)
        g = graph_ids[nodes] - c * GPC
        S[np.arange(nn), g] = 1.0
        sgts.append(_bf(S.reshape(NT, P, GPC).transpose(1, 0, 2)))

        ngs.append(_bf(np.bincount(g, minlength=GPC).reshape(1, GPC)))

    # ---- tile -> gather-instruction block mapping (uniform across cores)
    # seg job for tile t consumes blocks [blk0_t[t], blk0_t[t+1]) which live
    # in gather instr gi = blk//32 at local col blk%32.
    BPG = TOK_PER_G // P  # blocks per gather instr = 32

    # ---- protein branch: stacked-tap one-hot [76, 1000] bf16 per protein
    seq = np.asarray(inputs["protein_seq"]).reshape(NCORES, PPC, L)
    ohs = []
    for c in range(NCORES):
        oh = np.zeros((PPC, 76, L), np.float32)
        for t in range(3):
            jlo, jhi = max(0, 1 - t), min(L, L + 1 - t)
            # out position j uses seq[j + t - 1]
            sl = seq[c][:, jlo + t - 1: jhi + t - 1]
            oh[:, 25 * t:25 * t + VOCAB, jlo:jhi] = (
                sl[:, None, :] == np.arange(VOCAB)[None, :, None])
        oh[:, 75, :] = 1.0
        ohs.append(_bf(oh.reshape(PPC // 4, 4, 76, L).transpose(0, 2, 1, 3)))

    # ---- weights (folded, bf16)
    embed = np.asarray(inputs["embed"], np.float32)
    K1 = np.asarray(inputs["K1"], np.float32)   # [96, 128, 3]
    M1s = np.zeros((76, 96), np.float32)
    for t in range(3):
        M1s[25 * t:25 * t + VOCAB] = embed @ K1[:, :, t].T
    M1s[75] = np.asarray(inputs["cb1"], np.float32)

    def ktap(K, cb, cin, cout):
        # [cin+1, 3, cout]; ones-row bias on tap 0 only
        KT = np.zeros((cin + 1, 3, cout), np.float32)
        KT[:cin] = np.asarray(K, np.float32).transpose(1, 2, 0)
        KT[cin, 0, :] = np.asarray(cb, np.float32)
        return KT

    K2T = ktap(inputs["K2"], inputs["cb2"], 96, 128)
    K3T = np.ascontiguousarray(
        np.asarray(inputs["K3"], np.float32).transpose(1, 2, 0))  # [128,3,74]
    K4T = ktap(inputs["K4"], inputs["cb4"], 74, 128)

    W_gc_ext = np.zeros((75, HID), np.float32)
    W_gc_ext[:74] = np.asarray(inputs["W_gc"], np.float32)
    W_gc_ext[74] = np.asarray(inputs["b_gc"], np.float32)

    W_ri = np.asarray(inputs["W_ro_in"], np.float32)
    W_ro = np.asarray(inputs["W_ro_out"], np.float32)
    W_rr = W_ri @ W_ro
    b_rr = (np.asarray(inputs["b_ro_in"], np.float32) @ W_ro
            + np.asarray(inputs["b_ro_out"], np.float32))

    shared = {
        "M1s": _bf(M1s), "K2T": _bf(K2T), "K3T": _bf(K3T), "K4T": _bf(K4T),
        "cb3": np.asarray(inputs["cb3"], np.float32).reshape(IN_DIM, 1),
        "W_gc_ext": _bf(W_gc_ext),
        "W_rr": _bf(W_rr), "b_rr": _bf(b_rr.reshape(1, HID)),
        "Wc1": _bf(inputs["Wc1"]),
        "bc1": np.asarray(inputs["bc1"], np.float32).reshape(HID, 1),
        "Wc2": _bf(inputs["Wc2"]),
        "bc2": np.asarray(inputs["bc2"], np.float32).reshape(HID, 1),
        "Wf1_r": _bf(np.asarray(inputs["Wf1"], np.float32)
                     .reshape(2, HID, 2 * HID).transpose(1, 0, 2)),
        "bf1_r": np.ascontiguousarray(
            np.asarray(inputs["bf1"], np.float32).reshape(2, HID, 1)
            .transpose(1, 0, 2)),
        "Wf2_r": _bf(np.asarray(inputs["Wf2"], np.float32)
                     .reshape(2, HID, 1).transpose(1, 0, 2)),
        "bf2": np.asarray(inputs["bf2"], np.float32).reshape(1, 1),
        "ones1002": _bf(np.ones((1, 1002), np.float32)),
    }

    def wrap(a):  # token-major int16 -> wrapped [128, tokens//16]
        ncol = a.shape[0] // 16
        w = a.reshape(ncol, 16).T
        return np.ascontiguousarray(np.tile(w, (8, 1)))

    percore = []
    for c in range(NCORES):
        percore.append({
            "tab": tabs[c], "ix": wrap(idxs[c]), "M": Ms[c],
            "slab": slabs[c], "rdgi": rdgis[c], "Sgt": sgts[c],
            "ng": ngs[c], "onehot": ohs[c],
        })
    meta = dict(NT=NT, NBLK=NBLK, NBLK_PAD=NBLK_PAD, NG=NG, TOKS=TOKS,
                BPG=BPG, nblk_t=nblk_t.tolist(), blk0_t=blk0_t.tolist())
    return shared, percore, meta


# --------------------------------------------------------------- device build
def _build(shared, meta):
    NT, NBLK, NG, TOKS, BPG = (meta["NT"], meta["NBLK"], meta["NG"],
                               meta["TOKS"], meta["BPG"])
    NBLK_PAD = meta["NBLK_PAD"]
    nblk_t, blk0_t = meta["nblk_t"], meta["blk0_t"]

    nc = bacc.Bacc("TRN2", target_bir_lowering=False, debug=False,
                   num_devices=NCORES, num_swdge_queues=4)
    f32, bf16, i16 = dt.float32, dt.bfloat16, dt.int16

    D = {k: nc.dram_tensor(k, list(v.shape), dt.from_np(v.dtype),
                           kind="ExternalInput")
         for k, v in shared.items()}
    D["tab"] = nc.dram_tensor("tab", [32768, P], bf16, kind="ExternalInput")
    D["ix"] = nc.dram_tensor("ix", [P, TOKS // 16], i16, kind="ExternalInput")
    D["M"] = nc.dram_tensor("M", [P, NBLK_PAD, P], bf16,
                            kind="ExternalInput")
    D["slab"] = nc.dram_tensor("slab", [P, NT, P], bf16, kind="ExternalInput")
    D["rdgi"] = nc.dram_tensor("rdgi", [P, NT], f32, kind="ExternalInput")
    D["Sgt"] = nc.dram_tensor("Sgt", [P, NT, GPC], bf16, kind="ExternalInput")
    D["ng"] = nc.dram_tensor("ng", [1, GPC], bf16, kind="ExternalInput")
    D["onehot"] = nc.dram_tensor("onehot", [PPC // 4, 76, 4, L], bf16,
                                 kind="ExternalInput")
    out_d = nc.dram_tensor("out", [1, GPC], f32, kind="ExternalOutput")

    with tile.TileContext(nc) as tc, contextlib.ExitStack() as ctx:
        wp = ctx.enter_context(tc.tile_pool(name="wp", bufs=1))
        gp = ctx.enter_context(tc.tile_pool(name="gp", bufs=1))
        mp = ctx.enter_context(tc.tile_pool(name="mp", bufs=1))
        ohp = ctx.enter_context(tc.tile_pool(name="ohp", bufs=1))
        gnp = ctx.enter_context(tc.tile_pool(name="gnp", bufs=3))
        pcv = ctx.enter_context(tc.tile_pool(name="pcv", bufs=1, space="PSUM"))
        pg = ctx.enter_context(tc.tile_pool(name="pg", bufs=2, space="PSUM"))
        phg = ctx.enter_context(tc.tile_pool(name="phg", bufs=1, space="PSUM"))

        # ---------------- setup: weights to SBUF
        def ld(name, shape, dtype=bf16):
            t = wp.tile(shape, dtype, tag=name)
            nc.sync.dma_start(out=t[:], in_=D[name][:])
            return t

        M1s = ld("M1s", [76, 96])
        K2T = ld("K2T", [97, 3, HID])
        K3T = ld("K3T", [HID, 3, IN_DIM])
        K4T = ld("K4T", [75, 3, HID])
        cb3 = ld("cb3", [IN_DIM, 1], f32)
        Wgc = ld("W_gc_ext", [75, HID])
        Wrr = ld("W_rr", [HID, HID])
        brr = ld("b_rr", [1, HID])
        Wc1 = ld("Wc1", [HID, HID]); bc1 = ld("bc1", [HID, 1], f32)
        Wc2 = ld("Wc2", [HID, HID]); bc2 = ld("bc2", [HID, 1], f32)
        Wf1 = ld("Wf1_r", [HID, 2, 2 * HID])
        bf1 = ld("bf1_r", [HID, 2, 1], f32)
        Wf2 = ld("Wf2_r", [HID, 2, 1])
        bf2 = ld("bf2", [1, 1], f32)
        ixt = ld("ix", [P, TOKS // 16], i16)
        rdgi = ld("rdgi", [P, NT], f32)
        Sgt = ld("Sgt", [P, NT, GPC])
        ngt = ld("ng", [1, GPC])
        slab = ld("slab", [P, NT, P])

        identf = wp.tile([P, P], f32, tag="identf")
        make_identity(nc, identf[:])
        ident = wp.tile([P, P], bf16, tag="ident")
        nc.vector.tensor_copy(ident[:], identf[:])

        # conv activation buffers (2-protein rotation), ones rows + guards
        CIN1, CIN2, CIN3 = 97, 128, 75
        xb1, xb2, xb3 = [], [], []
        for j in range(2):
            t1 = wp.tile([CIN1, 1002], bf16, tag="xb1_%d" % j)
            nc.vector.memset(t1[:, 0:1], 0.0)
            nc.vector.memset(t1[:, 1001:1002], 0.0)
            nc.sync.dma_start(out=t1[96:97, :], in_=D["ones1002"][:])
            xb1.append(t1)
            t2 = wp.tile([CIN2, 1002], bf16, tag="xb2_%d" % j)
            nc.vector.memset(t2[:, 0:1], 0.0)
            nc.vector.memset(t2[:, 1001:1002], 0.0)
            xb2.append(t2)
            t3 = wp.tile([CIN3, 1002], bf16, tag="xb3_%d" % j)
            nc.vector.memset(t3[:, 0:1], 0.0)
            nc.vector.memset(t3[:, 1001:1002], 0.0)
            nc.sync.dma_start(out=t3[74:75, :], in_=D["ones1002"][:])
            xb3.append(t3)

        chunkmax = wp.tile([P, 2, PPC], f32, tag="chunkmax")
        pmax = wp.tile([P, PPC], bf16, tag="pmax")

        # aggT buffers with ones row (manual 3-buf rotation)
        aggTb = []
        for j in range(3):
            a = wp.tile([75, P], bf16, tag="aggT_%d" % j)
            nc.sync.dma_start(out=a[74:75, :], in_=D["ones1002"][:1, :P])
            aggTb.append(a)

        # ---------------- gather + M-group DMAs (paced in the main loop:
        # emitting all upfront deadlocks the sync queue — M(g)'s WAR wait on
        # the buffer of M(g-4) would block onehot DMAs queued behind it)
        gbufs, mbufs = [], []

        def emit_group(g):
            mt = mp.tile([P, BPG, P], bf16, name="mt", tag="m%d" % (g % 4))
            nc.sync.dma_start(out=mt[:],
                              in_=D["M"][:, g * BPG:(g + 1) * BPG, :])
            mbufs.append(mt)
            gt = gp.tile([P, BPG, P], bf16, name="gt", tag="g%d" % (g % 4))
            nc.gpsimd.dma_gather(
                out_ap=gt[:], in_ap=D["tab"][:],
                idxs_ap=ixt[:, g * (TOK_PER_G // 16):
                            (g + 1) * (TOK_PER_G // 16)],
                num_idxs=TOK_PER_G, num_idxs_reg=TOK_PER_G, elem_size=P,
                single_packet=False, queue_num=g % 4)
            gbufs.append(gt)

        # last tile that consumes blocks of group j (consumers of the buffer
        # that group j+4 reuses)
        def tile_of_blk(b):
            t = 0
            while t < NT - 1 and blk0_t[t + 1] <= b:
                t += 1
            return t
        t_last = [tile_of_blk(min(NBLK, (j + 1) * BPG) - 1)
                  for j in range(NG)]

        # ---------------- GNN tile job
        hgsum = phg.tile([GPC, HID], f32, space="PSUM", tag="hgsum")

        def tile_job(t):
            gi_last = (blk0_t[t + 1] - 1) // BPG
            while len(gbufs) <= gi_last:
                emit_group(next_g[0])
                next_g[0] += 1
            sp = pg.tile([P, P], f32, space="PSUM", tag="pgt")
            for i in range(nblk_t[t]):
                b = blk0_t[t] + i
                gi, bl = b // BPG, b % BPG
                nc.tensor.matmul(sp[:, :IN_DIM], mbufs[gi][:, bl, :],
                                 gbufs[gi][:, bl, :IN_DIM],
                                 start=(i == 0), stop=(i == nblk_t[t] - 1))
            acc = gnp.tile([P, IN_DIM], bf16, tag="acc")
            nc.vector.tensor_scalar_mul(acc[:], sp[:, :IN_DIM],
                                        rdgi[:, t:t + 1])
            nc.vector.tensor_tensor(out=acc[:], in0=acc[:],
                                    in1=slab[:, t, :IN_DIM], op=ALU.add)
            tp = pg.tile([P, P], bf16, space="PSUM", tag="pgt")
            nc.tensor.transpose(tp[:IN_DIM, :], acc[:], ident[:])
            aggT = aggTb[t % 3]
            nc.scalar.copy(aggT[:IN_DIM, :], tp[:IN_DIM, :])
            hp = pg.tile([P, P], f32, space="PSUM", tag="pgt")
            nc.tensor.matmul(hp[:], aggT[:], Wgc[:], start=True, stop=True)
            hT = gnp.tile([P, HID], bf16, tag="hT")
            nc.scalar.activation(hT[:], hp[:], AF.Relu)
            nc.tensor.matmul(hgsum[:], Sgt[:, t, :], hT[:],
                             start=(t == 0), stop=(t == NT - 1),
                             skip_group_check=True)

        # ---------------- conv protein pair
        def conv_pair(p0):
            grp = p0 // 4
            if p0 % 4 == 0:
                oh = ohp.tile([76, 4, L], bf16, tag="oh%d" % (grp % 2))
                nc.sync.dma_start(out=oh[:], in_=D["onehot"][grp])
                conv_pair.oh = oh
            oh = conv_pair.oh
            ps = {}
            # layer 1: single stacked matmul per chunk
            for p in (p0, p0 + 1):
                for cc in range(2):
                    pp = pcv.tile([96, 500], f32, space="PSUM", name="cps",
                                  tag="c%d" % ((p % 2) * 2 + cc))
                    nc.tensor.matmul(pp[:], M1s[:],
                                     oh[:, p % 4, cc * 500:cc * 500 + 500],
                                     start=True, stop=True)
                    ps[(p, cc)] = pp
            for p in (p0, p0 + 1):
                for cc in range(2):
                    dst = xb1[p % 2][0:96, 1 + cc * 500:501 + cc * 500]
                    if cc == 0:
                        nc.scalar.activation(dst, ps[(p, cc)][:], AF.Relu)
                    else:
                        nc.vector.tensor_scalar_max(dst, ps[(p, cc)][:], 0.0)
            # layers 2..4, tap-outer over the 4 chunk-jobs
            for l, (KT, cin, cout, xin, xout) in enumerate((
                    (K2T, 97, 128, xb1, xb2),
                    (K3T, 128, 74, xb2, xb3),
                    (K4T, 75, 128, xb3, None))):
                for tap in range(3):
                    for p in (p0, p0 + 1):
                        for cc in range(2):
                            tag = "c%d" % ((p % 2) * 2 + cc)
                            if tap == 0:
                                ps[(p, cc)] = pcv.tile(
                                    [cout, 500], f32, space="PSUM",
                                    name="cps", tag=tag)
                            nc.tensor.matmul(
                                ps[(p, cc)][:], KT[:cin, tap, :],
                                xin[p % 2][0:cin,
                                           cc * 500 + tap:cc * 500 + tap + 500],
                                start=(tap == 0), stop=(tap == 2))
                for p in (p0, p0 + 1):
                    for cc in range(2):
                        pp = ps[(p, cc)]
                        if xout is None:  # layer 4 -> maxpool
                            nc.vector.reduce_max(
                                out=chunkmax[:, cc, p:p + 1],
                                in_=pp[:, :500], axis=AX.X)
                        elif l == 1:      # layer 3: bias+relu fused on DVE
                            nc.vector.tensor_scalar(
                                out=xout[p % 2][0:cout,
                                                1 + cc * 500:501 + cc * 500],
                                in0=pp[:], scalar1=cb3[:], scalar2=0.0,
                                op0=ALU.add, op1=ALU.max)
                        else:             # layer 2
                            dst = xout[p % 2][0:cout,
                                              1 + cc * 500:501 + cc * 500]
                            if cc == 0:
                                nc.scalar.activation(dst, pp[:], AF.Relu)
                            else:
                                nc.vector.tensor_scalar_max(dst, pp[:], 0.0)

        # ---------------- main loop: 32 pairs, 2 tile jobs per pair
        tiles = list(range(NT))
        next_g = [0]

        def emit_ready_groups(k):
            while next_g[0] < NG and (
                    next_g[0] < 4
                    or t_last[next_g[0] - 4] // 2 + 1 <= k):
                emit_group(next_g[0])
                next_g[0] += 1

        for k in range(PPC // 2):
            emit_ready_groups(k)
            conv_pair(2 * k)
            for _ in range(2):
                if tiles:
                    tile_job(tiles.pop(0))
        while next_g[0] < NG:
            emit_group(next_g[0])
            next_g[0] += 1
        while tiles:
            tile_job(tiles.pop(0))

        # ---------------- maxpool finish
        mxt = wp.tile([P, PPC], f32, tag="mxt")
        nc.vector.tensor_reduce(out=mxt[:],
                                in_=chunkmax[:].rearrange("p c q -> p q c"),
                                axis=AX.X, op=ALU.max)
        nc.scalar.activation(pmax[:], mxt[:], AF.Relu)

        # ---------------- graph tail
        hgs = wp.tile([GPC, HID], bf16, tag="hgs")
        nc.scalar.copy(hgs[:], hgsum[:])
        hgt_ps = pg.tile([HID, GPC], bf16, space="PSUM", tag="pgt")
        nc.tensor.transpose(hgt_ps[:], hgs[:], ident[:GPC, :GPC])
        hgT = wp.tile([HID, GPC], bf16, tag="hgT")
        nc.scalar.copy(hgT[:], hgt_ps[:])
        rp = pg.tile([HID, GPC], f32, space="PSUM", tag="pgt")
        nc.tensor.matmul(rp[:], Wrr[:], hgT[:], start=True, stop=False)
        nc.tensor.matmul(rp[:], brr[:], ngt[:], start=False, stop=True)
        hg = wp.tile([HID, GPC], bf16, tag="hg")
        nc.scalar.activation(hg[:], rp[:], AF.Relu)
        c1p = pg.tile([HID, GPC], f32, space="PSUM", tag="pgt")
        nc.tensor.matmul(c1p[:], Wc1[:], hg[:], start=True, stop=True)
        cv1 = wp.tile([HID, GPC], bf16, tag="cv1")
        nc.scalar.activation(cv1[:], c1p[:], AF.Relu, bias=bc1[:])
        c2p = pg.tile([HID, GPC], f32, space="PSUM", tag="pgt")
        nc.tensor.matmul(c2p[:], Wc2[:], cv1[:], start=True, stop=True)
        cv2 = wp.tile([HID, GPC], bf16, tag="cv2")
        nc.scalar.activation(cv2[:], c2p[:], AF.Relu, bias=bc2[:])
        # head
        zin = [cv2, pmax]
        z2 = []
        for mc in range(2):
            zps = pg.tile([HID, GPC], f32, space="PSUM", tag="pgt")
            for kc in range(2):
                nc.tensor.matmul(zps[:], Wf1[:, kc, mc * HID:(mc + 1) * HID],
                                 zin[kc][:, :GPC], start=(kc == 0),
                                 stop=(kc == 1))
            zt = wp.tile([HID, GPC], bf16, tag="z2_%d" % mc)
            nc.scalar.activation(zt[:], zps[:], AF.Relu, bias=bf1[:, mc, :])
            z2.append(zt)
        ops = pg.tile([1, GPC], f32, space="PSUM", tag="pgt")
        for kc in range(2):
            nc.tensor.matmul(ops[:], Wf2[:, kc, :], z2[kc][:],
                             start=(kc == 0), stop=(kc == 1))
        ot = wp.tile([1, GPC], f32, tag="ot")
        nc.scalar.activation(ot[:], ops[:], AF.Sigmoid, bias=bf2[:1, :])
        nc.sync.dma_start(out=out_d[:], in_=ot[:])

    nc.compile()
    return nc


def kernel(**inputs):
    shared, percore, meta = _host_prep(inputs)
    nc = _build(shared, meta)
    in_maps = []
    for c in range(NCORES):
        m = dict(shared)
        m.update(percore[c])
        in_maps.append(m)
    res = run_bass_kernel_spmd(nc, in_maps, list(range(NCORES)))
    out = np.concatenate([res.results[c]["out"].reshape(GPC)
                          for c in range(NCORES)])
    return out.reshape(B, 1).astype(np.float32)


if __name__ == "__main__":
    sys.path.insert(0, "/root/problem")
    import jax
    import reference
    with jax.default_device(jax.devices("cpu")[0]):
        inputs = {k: np.asarray(v) for k, v in reference.setup_inputs().items()}
        exp = np.asarray(reference.reference(**inputs))
    got = kernel(**inputs)
    err = np.abs(got - exp).max()
    rel = err / max(np.abs(exp).max(), 1e-9)
    print("max abs err:", err, " rel:", rel)


# revision 25
# speedup vs baseline: 3.9403x; 1.0737x over previous
"""CPI_DGLLife kernel for 8 Trainium2 NeuronCores (SPMD), v2.

GCN over a 65536-node graph + protein conv1d branch + CPI head.
Sharding: data-parallel over the 512-graph batch (64 graphs / core).

v2 design (all matmuls bf16):
- conv layer 1 via stacked-tap one-hot (75-row rhs, 1 matmul per chunk);
  biases folded into matmuls via ones-rows (except layer 3, fused on DVE).
- GCN aggregation: dense dma_gather from a per-core compacted src table
  (rsqrt(deg_out) prescaled), segment-sum via one-hot M-matrix matmuls,
  self-loops added from a contiguous slab via DVE.
- node readout (two affine layers, no relu) folded into one matrix on
  host and applied per-graph after the S-matmul sum.
"""
import sys
sys.path.insert(0, "/opt/trn_rl_repo")
import contextlib
import numpy as np
import ml_dtypes

import concourse.bass as bass
import concourse.bacc as bacc
import concourse.tile as tile
from concourse import mybir
from concourse.bass_utils import run_bass_kernel_spmd
from concourse.masks import make_identity

dt = mybir.dt
AF = mybir.ActivationFunctionType
ALU = mybir.AluOpType
AX = mybir.AxisListType

P = 128
N, E, B, L = 65536, 262144, 512, 1000
IN_DIM, HID, VOCAB = 74, 128, 25
CHANNELS = [HID, 96, 128, IN_DIM, HID]
NCORES = 8
GPC = B // NCORES              # graphs per core = 64
PPC = GPC                      # proteins per core = 64
TOK_PER_G = 4096               # tokens per dma_gather instruction
BF16 = ml_dtypes.bfloat16


def _bf(a):
    return np.ascontiguousarray(np.asarray(a, np.float32).astype(BF16))


# ------------------------------------------------------------------ host prep
def _host_prep(inputs):
    graph_ids = np.asarray(inputs["graph_ids"])
    esrc = np.asarray(inputs["edge_src"]).astype(np.int64)
    edst = np.asarray(inputs["edge_dst"]).astype(np.int64)
    src_all = np.concatenate([esrc, np.arange(N, dtype=np.int64)])
    dst_all = np.concatenate([edst, np.arange(N, dtype=np.int64)])
    deg_out = np.bincount(src_all, minlength=N).astype(np.float32)
    deg_in = np.bincount(dst_all, minlength=N).astype(np.float32)
    rs_out = 1.0 / np.sqrt(deg_out)
    rs_in = 1.0 / np.sqrt(deg_in)

    nf = np.asarray(inputs["node_feats"], np.float32)
    nf_scaled = nf * rs_out[:, None]           # rows for gather table

    core_node_lo = np.searchsorted(graph_ids, np.arange(0, B + 1, GPC))
    ncore_nodes = core_node_lo[1:] - core_node_lo[:-1]
    NT = int(np.ceil(ncore_nodes.max() / P))
    NPAD = NT * P

    # ---- per-core edge streams (natural dst order), compacted src tables
    ecore = np.searchsorted(core_node_lo[1:], edst, side="right")
    order = np.argsort(ecore * np.int64(N) + edst, kind="stable")
    es, ed, ec = esrc[order], edst[order], ecore[order]
    core_e_lo = np.searchsorted(ec, np.arange(NCORES + 1))

    # per (core, tile) counts -> uniform nblk per tile (max over cores)
    cnt = np.zeros((NCORES, NT), np.int64)
    for c in range(NCORES):
        lo, hi = core_e_lo[c], core_e_lo[c + 1]
        pos = ed[lo:hi] - core_node_lo[c]
        np.add.at(cnt[c], pos // P, 1)
    nblk_t = np.maximum(1, np.ceil(cnt.max(axis=0) / P).astype(np.int64))
    NBLK = int(nblk_t.sum())
    TOKS = NBLK * P
    NG = int(np.ceil(TOKS / TOK_PER_G))
    NBLK_PAD = NBLK
    blk0_t = np.concatenate([[0], np.cumsum(nblk_t)])  # block offset per tile

    tabs, idxs, Ms, slabs, rdgis, sgts, ngs = [], [], [], [], [], [], []
    TABROWS = 32768
    for c in range(NCORES):
        lo, hi = core_e_lo[c], core_e_lo[c + 1]
        s, d = es[lo:hi], ed[lo:hi] - core_node_lo[c]
        uniq, inv = np.unique(s, return_inverse=True)
        assert len(uniq) + 1 <= TABROWS, len(uniq)
        tab = np.zeros((TABROWS, P), np.float32)
        tab[1:1 + len(uniq), :IN_DIM] = nf_scaled[uniq]
        tabs.append(_bf(tab))

        # token stream: per tile, its edges at block offset blk0_t[t]
        tok_idx = np.zeros(TOKS, np.int16)
        Mm = np.zeros((NBLK * P,), np.int64)  # dst lane per token (-1 = pad)
        Mm[:] = -1
        t_of = d // P
        lane = d % P
        # position within tile = running index among that tile's edges
        o2 = np.argsort(t_of, kind="stable")
        t_sorted = t_of[o2]
        starts = np.searchsorted(t_sorted, np.arange(NT))
        for t in range(NT):
            a, b = starts[t], starts[t] + cnt[c][t]
            sel = o2[a:b]
            base = blk0_t[t] * P
            tok_idx[base:base + len(sel)] = (inv[sel] + 1).astype(np.int16)
            Mm[base:base + len(sel)] = lane[sel]
        idxs.append(tok_idx)

        # M matrices [128 tok, NBLK_PAD, 128 dst] bf16 (token-part. major)
        M = np.zeros((P, NBLK_PAD, P), np.float32)
        tok = np.arange(NBLK * P)
        valid = Mm >= 0
        M[tok[valid] % P, tok[valid] // P, Mm[valid]] = 1.0
        Ms.append(_bf(M))

        # self slab [128, NT, 128]: X[n]*rs_out[n]*rs_in[n] in natural order
        slab = np.zeros((NPAD, P), np.float32)
        nn = int(ncore_nodes[c])
        nodes = np.arange(core_node_lo[c], core_node_lo[c] + nn)
        slab[:nn, :IN_DIM] = nf[nodes] * (rs_out[nodes] * rs_in[nodes])[:, None]
        slabs.append(_bf(slab.reshape(NT, P, P).transpose(1, 0, 2)))

        rdgi = np.ones((NPAD,), np.float32)
        rdgi[:nn] = rs_in[nodes]
        rdgis.append(np.ascontiguousarray(
            rdgi.reshape(NT, P).T.astype(np.float32)))

        # S^T tiles [128 node-lane, NT, 64 graphs]
        S = np.zeros((NPAD, GPC), np.float32)
        g = graph_ids[nodes] - c * GPC
        S[np.arange(nn), g] = 1.0
        sgts.append(_bf(S.reshape(NT, P, GPC).transpose(1, 0, 2)))

        ngs.append(_bf(np.bincount(g, minlength=GPC).reshape(1, GPC)))

    # ---- tile -> gather-instruction block mapping (uniform across cores)
    # seg job for tile t consumes blocks [blk0_t[t], blk0_t[t+1]) which live
    # in gather instr gi = blk//32 at local col blk%32.
    BPG = TOK_PER_G // P  # blocks per gather instr = 32

    # ---- protein branch: stacked-tap one-hot [76, 1000] bf16 per protein
    seq = np.asarray(inputs["protein_seq"]).reshape(NCORES, PPC, L)
    ohs = []
    for c in range(NCORES):
        oh = np.zeros((PPC, 76, L), np.float32)
        for t in range(3):
            jlo, jhi = max(0, 1 - t), min(L, L + 1 - t)
            # out position j uses seq[j + t - 1]
            sl = seq[c][:, jlo + t - 1: jhi + t - 1]
            oh[:, 25 * t:25 * t + VOCAB, jlo:jhi] = (
                sl[:, None, :] == np.arange(VOCAB)[None, :, None])
        oh[:, 75, :] = 1.0
        ohs.append(_bf(oh.reshape(PPC // 4, 4, 76, L).transpose(0, 2, 1, 3)))

    # ---- weights (folded, bf16)
    embed = np.asarray(inputs["embed"], np.float32)
    K1 = np.asarray(inputs["K1"], np.float32)   # [96, 128, 3]
    M1s = np.zeros((76, 96), np.float32)
    for t in range(3):
        M1s[25 * t:25 * t + VOCAB] = embed @ K1[:, :, t].T
    M1s[75] = np.asarray(inputs["cb1"], np.float32)

    def ktap(K, cb, cin, cout):
        # [cin+1, 3, cout]; ones-row bias on tap 0 only
        KT = np.zeros((cin + 1, 3, cout), np.float32)
        KT[:cin] = np.asarray(K, np.float32).transpose(1, 2, 0)
        KT[cin, 0, :] = np.asarray(cb, np.float32)
        return KT

    K2T = ktap(inputs["K2"], inputs["cb2"], 96, 128)
    K3T = np.ascontiguousarray(
        np.asarray(inputs["K3"], np.float32).transpose(1, 2, 0))  # [128,3,74]
    K4T = ktap(inputs["K4"], inputs["cb4"], 74, 128)

    W_gc_ext = np.zeros((75, HID), np.float32)
    W_gc_ext[:74] = np.asarray(inputs["W_gc"], np.float32)
    W_gc_ext[74] = np.asarray(inputs["b_gc"], np.float32)

    W_ri = np.asarray(inputs["W_ro_in"], np.float32)
    W_ro = np.asarray(inputs["W_ro_out"], np.float32)
    W_rr = W_ri @ W_ro
    b_rr = (np.asarray(inputs["b_ro_in"], np.float32) @ W_ro
            + np.asarray(inputs["b_ro_out"], np.float32))

    shared = {
        "M1s": _bf(M1s), "K2T": _bf(K2T), "K3T": _bf(K3T), "K4T": _bf(K4T),
        "cb3": np.asarray(inputs["cb3"], np.float32).reshape(IN_DIM, 1),
        "W_gc_ext": _bf(W_gc_ext),
        "W_rr": _bf(W_rr), "b_rr": _bf(b_rr.reshape(1, HID)),
        "Wc1": _bf(inputs["Wc1"]),
        "bc1": np.asarray(inputs["bc1"], np.float32).reshape(HID, 1),
        "Wc2": _bf(inputs["Wc2"]),
        "bc2": np.asarray(inputs["bc2"], np.float32).reshape(HID, 1),
        "Wf1_r": _bf(np.asarray(inputs["Wf1"], np.float32)
                     .reshape(2, HID, 2 * HID).transpose(1, 0, 2)),
        "bf1_r": np.ascontiguousarray(
            np.asarray(inputs["bf1"], np.float32).reshape(2, HID, 1)
            .transpose(1, 0, 2)),
        "Wf2_r": _bf(np.asarray(inputs["Wf2"], np.float32)
                     .reshape(2, HID, 1).transpose(1, 0, 2)),
        "bf2": np.asarray(inputs["bf2"], np.float32).reshape(1, 1),
        "ones1002": _bf(np.ones((1, 1002), np.float32)),
    }

    def wrap(a):  # token-major int16 -> wrapped [128, tokens//16]
        ncol = a.shape[0] // 16
        w = a.reshape(ncol, 16).T
        return np.ascontiguousarray(np.tile(w, (8, 1)))

    percore = []
    for c in range(NCORES):
        percore.append({
            "tab": tabs[c], "ix": wrap(idxs[c]), "M": Ms[c],
            "slab": slabs[c], "rdgi": rdgis[c], "Sgt": sgts[c],
            "ng": ngs[c], "onehot": ohs[c],
        })
    meta = dict(NT=NT, NBLK=NBLK, NBLK_PAD=NBLK_PAD, NG=NG, TOKS=TOKS,
                BPG=BPG, nblk_t=nblk_t.tolist(), blk0_t=blk0_t.tolist())
    return shared, percore, meta


# --------------------------------------------------------------- device build
def _build(shared, meta):
    NT, NBLK, NG, TOKS, BPG = (meta["NT"], meta["NBLK"], meta["NG"],
                               meta["TOKS"], meta["BPG"])
    NBLK_PAD = meta["NBLK_PAD"]
    nblk_t, blk0_t = meta["nblk_t"], meta["blk0_t"]

    nc = bacc.Bacc("TRN2", target_bir_lowering=False, debug=False,
                   num_devices=NCORES, num_swdge_queues=4)
    f32, bf16, i16 = dt.float32, dt.bfloat16, dt.int16

    D = {k: nc.dram_tensor(k, list(v.shape), dt.from_np(v.dtype),
                           kind="ExternalInput")
         for k, v in shared.items()}
    D["tab"] = nc.dram_tensor("tab", [32768, P], bf16, kind="ExternalInput")
    D["ix"] = nc.dram_tensor("ix", [P, TOKS // 16], i16, kind="ExternalInput")
    D["M"] = nc.dram_tensor("M", [P, NBLK_PAD, P], bf16,
                            kind="ExternalInput")
    D["slab"] = nc.dram_tensor("slab", [P, NT, P], bf16, kind="ExternalInput")
    D["rdgi"] = nc.dram_tensor("rdgi", [P, NT], f32, kind="ExternalInput")
    D["Sgt"] = nc.dram_tensor("Sgt", [P, NT, GPC], bf16, kind="ExternalInput")
    D["ng"] = nc.dram_tensor("ng", [1, GPC], bf16, kind="ExternalInput")
    D["onehot"] = nc.dram_tensor("onehot", [PPC // 4, 76, 4, L], bf16,
                                 kind="ExternalInput")
    out_d = nc.dram_tensor("out", [1, GPC], f32, kind="ExternalOutput")

    with tile.TileContext(nc) as tc, contextlib.ExitStack() as ctx:
        wp = ctx.enter_context(tc.tile_pool(name="wp", bufs=1))
        gp = ctx.enter_context(tc.tile_pool(name="gp", bufs=1))
        mp = ctx.enter_context(tc.tile_pool(name="mp", bufs=1))
        ohp = ctx.enter_context(tc.tile_pool(name="ohp", bufs=1))
        gnp = ctx.enter_context(tc.tile_pool(name="gnp", bufs=3))
        pcv = ctx.enter_context(tc.tile_pool(name="pcv", bufs=1, space="PSUM"))
        pg = ctx.enter_context(tc.tile_pool(name="pg", bufs=2, space="PSUM"))
        phg = ctx.enter_context(tc.tile_pool(name="phg", bufs=1, space="PSUM"))

        # ---------------- setup: weights to SBUF
        def ld(name, shape, dtype=bf16):
            t = wp.tile(shape, dtype, tag=name)
            nc.sync.dma_start(out=t[:], in_=D[name][:])
            return t

        M1s = ld("M1s", [76, 96])
        K2T = ld("K2T", [97, 3, HID])
        K3T = ld("K3T", [HID, 3, IN_DIM])
        K4T = ld("K4T", [75, 3, HID])
        cb3 = ld("cb3", [IN_DIM, 1], f32)
        Wgc = ld("W_gc_ext", [75, HID])
        Wrr = ld("W_rr", [HID, HID])
        brr = ld("b_rr", [1, HID])
        Wc1 = ld("Wc1", [HID, HID]); bc1 = ld("bc1", [HID, 1], f32)
        Wc2 = ld("Wc2", [HID, HID]); bc2 = ld("bc2", [HID, 1], f32)
        Wf1 = ld("Wf1_r", [HID, 2, 2 * HID])
        bf1 = ld("bf1_r", [HID, 2, 1], f32)
        Wf2 = ld("Wf2_r", [HID, 2, 1])
        bf2 = ld("bf2", [1, 1], f32)
        ixt = ld("ix", [P, TOKS // 16], i16)
        rdgi = ld("rdgi", [P, NT], f32)
        Sgt = ld("Sgt", [P, NT, GPC])
        ngt = ld("ng", [1, GPC])
        slab = ld("slab", [P, NT, P])

        identf = wp.tile([P, P], f32, tag="identf")
        make_identity(nc, identf[:])
        ident = wp.tile([P, P], bf16, tag="ident")
        nc.vector.tensor_copy(ident[:], identf[:])

        # conv activation buffers (2-protein rotation), ones rows + guards
        CIN1, CIN2, CIN3 = 97, 128, 75
        xb1, xb2, xb3 = [], [], []
        for j in range(2):
            t1 = wp.tile([CIN1, 1002], bf16, tag="xb1_%d" % j)
            nc.vector.memset(t1[:, 0:1], 0.0)
            nc.vector.memset(t1[:, 1001:1002], 0.0)
            nc.sync.dma_start(out=t1[96:97, :], in_=D["ones1002"][:])
            xb1.append(t1)
            t2 = wp.tile([CIN2, 1002], bf16, tag="xb2_%d" % j)
            nc.vector.memset(t2[:, 0:1], 0.0)
            nc.vector.memset(t2[:, 1001:1002], 0.0)
            xb2.append(t2)
            t3 = wp.tile([CIN3, 1002], bf16, tag="xb3_%d" % j)
            nc.vector.memset(t3[:, 0:1], 0.0)
            nc.vector.memset(t3[:, 1001:1002], 0.0)
            nc.sync.dma_start(out=t3[74:75, :], in_=D["ones1002"][:])
            xb3.append(t3)

        chunkmax = wp.tile([P, 2, PPC], f32, tag="chunkmax")
        pmax = wp.tile([P, PPC], bf16, tag="pmax")

        # aggT buffers with ones row (manual 3-buf rotation)
        aggTb = []
        for j in range(3):
            a = wp.tile([75, P], bf16, tag="aggT_%d" % j)
            nc.sync.dma_start(out=a[74:75, :], in_=D["ones1002"][:1, :P])
            aggTb.append(a)

        # ---------------- gather + M-group DMAs (paced in the main loop:
        # emitting all upfront deadlocks the sync queue — M(g)'s WAR wait on
        # the buffer of M(g-4) would block onehot DMAs queued behind it)
        gbufs, mbufs = [], []

        NBUF = 6

        def emit_group(g):
            nb = min(BPG, NBLK - g * BPG)
            ntok = nb * P
            mt = mp.tile([P, nb, P], bf16, name="mt", tag="m%d" % (g % NBUF))
            nc.sync.dma_start(out=mt[:],
                              in_=D["M"][:, g * BPG:g * BPG + nb, :])
            mbufs.append(mt)
            gt = gp.tile([P, nb, P], bf16, name="gt", tag="g%d" % (g % NBUF))
            nc.gpsimd.dma_gather(
                out_ap=gt[:], in_ap=D["tab"][:],
                idxs_ap=ixt[:, g * (TOK_PER_G // 16):
                            g * (TOK_PER_G // 16) + ntok // 16],
                num_idxs=ntok, num_idxs_reg=ntok, elem_size=P,
                single_packet=False, queue_num=g % 4)
            gbufs.append(gt)

        # last tile that consumes blocks of group j (consumers of the buffer
        # that group j+4 reuses)
        def tile_of_blk(b):
            t = 0
            while t < NT - 1 and blk0_t[t + 1] <= b:
                t += 1
            return t
        t_last = [tile_of_blk(min(NBLK, (j + 1) * BPG) - 1)
                  for j in range(NG)]

        # ---------------- GNN tile job
        hgsum = phg.tile([GPC, HID], f32, space="PSUM", tag="hgsum")

        def tile_job(t):
            gi_last = (blk0_t[t + 1] - 1) // BPG
            while len(gbufs) <= gi_last:
                emit_group(next_g[0])
                next_g[0] += 1
            sp = pg.tile([P, P], f32, space="PSUM", tag="pgt")
            for i in range(nblk_t[t]):
                b = blk0_t[t] + i
                gi, bl = b // BPG, b % BPG
                nc.tensor.matmul(sp[:, :IN_DIM], mbufs[gi][:, bl, :],
                                 gbufs[gi][:, bl, :IN_DIM],
                                 start=(i == 0), stop=(i == nblk_t[t] - 1))
            acc = gnp.tile([P, IN_DIM], bf16, tag="acc")
            nc.vector.tensor_scalar_mul(acc[:], sp[:, :IN_DIM],
                                        rdgi[:, t:t + 1])
            nc.vector.tensor_tensor(out=acc[:], in0=acc[:],
                                    in1=slab[:, t, :IN_DIM], op=ALU.add)
            tp = pg.tile([P, P], bf16, space="PSUM", tag="pgt")
            nc.tensor.transpose(tp[:IN_DIM, :], acc[:], ident[:])
            aggT = aggTb[t % 3]
            nc.scalar.copy(aggT[:IN_DIM, :], tp[:IN_DIM, :])
            hp = pg.tile([P, P], f32, space="PSUM", tag="pgt")
            nc.tensor.matmul(hp[:], aggT[:], Wgc[:], start=True, stop=True)
            hT = gnp.tile([P, HID], bf16, tag="hT")
            nc.scalar.activation(hT[:], hp[:], AF.Relu)
            nc.tensor.matmul(hgsum[:], Sgt[:, t, :], hT[:],
                             start=(t == 0), stop=(t == NT - 1),
                             skip_group_check=True)

        # ---------------- conv protein pair
        def conv_pair(p0):
            grp = p0 // 4
            if p0 % 4 == 0:
                oh = ohp.tile([76, 4, L], bf16, tag="oh%d" % (grp % 2))
                nc.sync.dma_start(out=oh[:], in_=D["onehot"][grp])
                conv_pair.oh = oh
            oh = conv_pair.oh
            ps = {}
            # layer 1: single stacked matmul per chunk
            for p in (p0, p0 + 1):
                for cc in range(2):
                    pp = pcv.tile([96, 500], f32, space="PSUM", name="cps",
                                  tag="c%d" % ((p % 2) * 2 + cc))
                    nc.tensor.matmul(pp[:], M1s[:],
                                     oh[:, p % 4, cc * 500:cc * 500 + 500],
                                     start=True, stop=True)
                    ps[(p, cc)] = pp
            for p in (p0, p0 + 1):
                for cc in range(2):
                    dst = xb1[p % 2][0:96, 1 + cc * 500:501 + cc * 500]
                    nc.scalar.activation(dst, ps[(p, cc)][:], AF.Relu)
            # layers 2..4, tap-outer over the 4 chunk-jobs
            for l, (KT, cin, cout, xin, xout) in enumerate((
                    (K2T, 97, 128, xb1, xb2),
                    (K3T, 128, 74, xb2, xb3),
                    (K4T, 75, 128, xb3, None))):
                for tap in range(3):
                    for p in (p0, p0 + 1):
                        for cc in range(2):
                            tag = "c%d" % ((p % 2) * 2 + cc)
                            if tap == 0:
                                ps[(p, cc)] = pcv.tile(
                                    [cout, 500], f32, space="PSUM",
                                    name="cps", tag=tag)
                            nc.tensor.matmul(
                                ps[(p, cc)][:], KT[:cin, tap, :],
                                xin[p % 2][0:cin,
                                           cc * 500 + tap:cc * 500 + tap + 500],
                                start=(tap == 0), stop=(tap == 2))
                for p in (p0, p0 + 1):
                    for cc in range(2):
                        pp = ps[(p, cc)]
                        if xout is None:  # layer 4 -> maxpool
                            nc.vector.reduce_max(
                                out=chunkmax[:, cc, p:p + 1],
                                in_=pp[:, :500], axis=AX.X)
                        elif l == 1:      # layer 3: bias+relu fused on DVE
                            nc.vector.tensor_scalar(
                                out=xout[p % 2][0:cout,
                                                1 + cc * 500:501 + cc * 500],
                                in0=pp[:], scalar1=cb3[:], scalar2=0.0,
                                op0=ALU.add, op1=ALU.max)
                        else:             # layer 2
                            dst = xout[p % 2][0:cout,
                                              1 + cc * 500:501 + cc * 500]
                            nc.scalar.activation(dst, pp[:], AF.Relu)

        # ---------------- main loop: 32 pairs, 2 tile jobs per pair
        tiles = list(range(NT))
        next_g = [0]

        def emit_ready_groups(k):
            while next_g[0] < NG and (
                    next_g[0] < NBUF
                    or t_last[next_g[0] - NBUF] // 2 + 1 <= k):
                emit_group(next_g[0])
                next_g[0] += 1

        for k in range(PPC // 2):
            emit_ready_groups(k)
            conv_pair(2 * k)
            for _ in range(2):
                if tiles:
                    tile_job(tiles.pop(0))
        while next_g[0] < NG:
            emit_group(next_g[0])
            next_g[0] += 1
        while tiles:
            tile_job(tiles.pop(0))

        # ---------------- maxpool finish
        mxt = wp.tile([P, PPC], f32, tag="mxt")
        nc.vector.tensor_reduce(out=mxt[:],
                                in_=chunkmax[:].rearrange("p c q -> p q c"),
                                axis=AX.X, op=ALU.max)
        nc.scalar.activation(pmax[:], mxt[:], AF.Relu)

        # ---------------- graph tail
        hgs = wp.tile([GPC, HID], bf16, tag="hgs")
        nc.scalar.copy(hgs[:], hgsum[:])
        hgt_ps = pg.tile([HID, GPC], bf16, space="PSUM", tag="pgt")
        nc.tensor.transpose(hgt_ps[:], hgs[:], ident[:GPC, :GPC])
        hgT = wp.tile([HID, GPC], bf16, tag="hgT")
        nc.scalar.copy(hgT[:], hgt_ps[:])
        rp = pg.tile([HID, GPC], f32, space="PSUM", tag="pgt")
        nc.tensor.matmul(rp[:], Wrr[:], hgT[:], start=True, stop=False)
        nc.tensor.matmul(rp[:], brr[:], ngt[:], start=False, stop=True)
        hg = wp.tile([HID, GPC], bf16, tag="hg")
        nc.scalar.activation(hg[:], rp[:], AF.Relu)
        c1p = pg.tile([HID, GPC], f32, space="PSUM", tag="pgt")
        nc.tensor.matmul(c1p[:], Wc1[:], hg[:], start=True, stop=True)
        cv1 = wp.tile([HID, GPC], bf16, tag="cv1")
        nc.scalar.activation(cv1[:], c1p[:], AF.Relu, bias=bc1[:])
        c2p = pg.tile([HID, GPC], f32, space="PSUM", tag="pgt")
        nc.tensor.matmul(c2p[:], Wc2[:], cv1[:], start=True, stop=True)
        cv2 = wp.tile([HID, GPC], bf16, tag="cv2")
        nc.scalar.activation(cv2[:], c2p[:], AF.Relu, bias=bc2[:])
        # head
        zin = [cv2, pmax]
        z2 = []
        for mc in range(2):
            zps = pg.tile([HID, GPC], f32, space="PSUM", tag="pgt")
            for kc in range(2):
                nc.tensor.matmul(zps[:], Wf1[:, kc, mc * HID:(mc + 1) * HID],
                                 zin[kc][:, :GPC], start=(kc == 0),
                                 stop=(kc == 1))
            zt = wp.tile([HID, GPC], bf16, tag="z2_%d" % mc)
            nc.scalar.activation(zt[:], zps[:], AF.Relu, bias=bf1[:, mc, :])
            z2.append(zt)
        ops = pg.tile([1, GPC], f32, space="PSUM", tag="pgt")
        for kc in range(2):
            nc.tensor.matmul(ops[:], Wf2[:, kc, :], z2[kc][:],
                             start=(kc == 0), stop=(kc == 1))
        ot = wp.tile([1, GPC], f32, tag="ot")
        nc.scalar.activation(ot[:], ops[:], AF.Sigmoid, bias=bf2[:1, :])
        nc.sync.dma_start(out=out_d[:], in_=ot[:])

    nc.compile()
    return nc


def kernel(**inputs):
    shared, percore, meta = _host_prep(inputs)
    nc = _build(shared, meta)
    in_maps = []
    for c in range(NCORES):
        m = dict(shared)
        m.update(percore[c])
        in_maps.append(m)
    res = run_bass_kernel_spmd(nc, in_maps, list(range(NCORES)))
    out = np.concatenate([res.results[c]["out"].reshape(GPC)
                          for c in range(NCORES)])
    return out.reshape(B, 1).astype(np.float32)


if __name__ == "__main__":
    sys.path.insert(0, "/root/problem")
    import jax
    import reference
    with jax.default_device(jax.devices("cpu")[0]):
        inputs = {k: np.asarray(v) for k, v in reference.setup_inputs().items()}
        exp = np.asarray(reference.reference(**inputs))
    got = kernel(**inputs)
    err = np.abs(got - exp).max()
    rel = err / max(np.abs(exp).max(), 1e-9)
    print("max abs err:", err, " rel:", rel)
